# revision 2
# baseline (speedup 1.0000x reference)
"""Criss-cross attention (CCNet) kernel for 8 TRN2 NeuronCores.

Data-parallel over batch N=8: one image per core. Per image (512ch, 96x96):
  t/f = 1x1 conv to 64ch; g = 1x1 conv to 512ch
  row/col affinities -> softmax over 191 (96 row + 95 col, col diag excluded)
  weighted row/col aggregation of g -> inc 1x1 conv -> residual add.

All matmuls bf16 operands with f32 PSUM accumulation. Weight transposes and
bf16 casts are done on host (numpy) - they are kernel inputs.
"""

import sys

sys.path.insert(0, "/opt/trn_rl_repo")

from contextlib import ExitStack

import numpy as np
import ml_dtypes

import concourse.bass as bass
import concourse.bacc as bacc
import concourse.tile as tile
from concourse import mybir
from concourse.bass_utils import run_bass_kernel_spmd

BF16 = mybir.dt.bfloat16
F32 = mybir.dt.float32
AF = mybir.ActivationFunctionType

N, C_IN, C_INNER, C_OUT, H, W = 8, 512, 64, 512, 96, 96
HW = H * W  # 9216
KC = C_IN // 128  # 4 contraction chunks

_cache = {}


def build_program():
    nc = bacc.Bacc()

    # ---- DRAM I/O ----
    xbf_d = nc.dram_tensor("x_bf", (128, KC, HW), BF16, kind="ExternalInput")
    xf_d = nc.dram_tensor("x_f32", (KC, 128, HW), F32, kind="ExternalInput")
    tfw_d = nc.dram_tensor("tf_wT", (128, KC, 128), BF16, kind="ExternalInput")
    tfw2_d = nc.dram_tensor("tf_wT2", (128, KC, 128), BF16, kind="ExternalInput")
    gw_d = nc.dram_tensor("g_wT", (128, KC, C_OUT), BF16, kind="ExternalInput")
    incw_d = nc.dram_tensor("inc_wT", (128, KC, C_IN), BF16, kind="ExternalInput")
    tfb_d = nc.dram_tensor("tf_b", (128, 1), F32, kind="ExternalInput")
    combb_d = nc.dram_tensor("comb_b", (128, KC), F32, kind="ExternalInput")
    tfb2_d = nc.dram_tensor("tf_b2", (128, 1), F32, kind="ExternalInput")
    mask_d = nc.dram_tensor("mask", (96, 96), BF16, kind="ExternalInput")
    ones96b_d = nc.dram_tensor("ones96b", (96, 128), BF16, kind="ExternalInput")
    out_d = nc.dram_tensor("out", (KC, 128, HW), F32, kind="ExternalOutput")

    with ExitStack() as ctx:
        tc = ctx.enter_context(tile.TileContext(nc))
        p0 = ctx.enter_context(tc.tile_pool(name="p0", bufs=1))

        # ---- persistent tiles ----
        Xbf = p0.tile([128, KC, H, W], BF16)  # channel-major image, bf16
        ones96b = p0.tile([96, 128], BF16)
        mask = p0.tile([96, 96], BF16)
        gw = p0.tile([128, KC, C_OUT], BF16)

        nc.sync.dma_start(out=ones96b, in_=ones96b_d[:])
        nc.sync.dma_start(out=mask, in_=mask_d[:])
        nc.sync.dma_start(out=gw, in_=gw_d[:])
        xv = xbf_d[:].rearrange("p a (h w) -> p a h w", h=H)

        # T/F (phase 1) and U (phases 3-4) share one big slot: disjoint lifetimes
        TFb = p0.tile([128, 2, H, W], BF16, tag="big", name="TFb")
        T = TFb[0:64, 0]
        F = TFb[0:64, 1]
        T2 = TFb[64:128, 0]
        F2 = TFb[64:128, 1]

        with tc.tile_pool(name="pwr", bufs=1) as pwr:
            # exp(affinity) buffers: Wr[i, y, x] (row), Wc[j, x, y] (col)
            Wr = pwr.tile([96, H, W], BF16)
            with tc.tile_pool(name="pwc", bufs=1) as pwc:
                Wc = pwc.tile([96, W, H], BF16)

                # ---- phase 1: t/f conv + affinities ----
                with tc.tile_pool(name="pe", bufs=1) as pe, \
                     tc.tile_pool(name="pe_ps", bufs=2, space="PSUM") as pe_ps, \
                     tc.tile_pool(name="ptf_ps", bufs=2, space="PSUM") as ptf_ps:
                    tfw = pe.tile([128, KC, 128], BF16)
                    tfw2 = pe.tile([128, KC, 128], BF16)
                    tb = pe.tile([64, 1], F32)
                    fb = pe.tile([64, 1], F32)
                    tfbF = pe.tile([128, 1], F32)
                    tb2 = pe.tile([128, 1], F32)
                    nc.sync.dma_start(out=tfw, in_=tfw_d[:])
                    nc.sync.dma_start(out=tfw2, in_=tfw2_d[:])
                    nc.sync.dma_start(out=tb, in_=tfb_d[0:64])
                    nc.sync.dma_start(out=fb, in_=tfb_d[64:128])
                    nc.sync.dma_start(out=tfbF, in_=tfb_d[:])
                    nc.sync.dma_start(out=tb2, in_=tfb2_d[:])
                    for q in range(8):
                        for k in range(KC):
                            nc.sync.dma_start(
                                out=Xbf[:, k, q * 12:(q + 1) * 12, :],
                                in_=xv[:, k, q * 12:(q + 1) * 12, :])

                    Xflat = Xbf.rearrange("p a h w -> p a (h w)")
                    Tflat = TFb.rearrange("p c h w -> p c (h w)")
                    for b in range(HW // 512):
                        sl = slice(b * 512, (b + 1) * 512)
                        pst = ptf_ps.tile([128, 512], F32, tag="pt")
                        psf = ptf_ps.tile([128, 512], F32, tag="pf")
                        for k in range(KC):
                            nc.tensor.matmul(
                                pst, tfw[:, k, :], Xflat[:, k, sl],
                                start=(k == 0), stop=(k == KC - 1))
                        for k in range(KC):
                            nc.tensor.matmul(
                                psf, tfw2[:, k, :], Xflat[:, k, sl],
                                start=(k == 0), stop=(k == KC - 1))
                        # pst = [t; f] stacked, psf = [f; t] stacked
                        nc.scalar.activation(Tflat[0:64, 0, sl], pst[0:64],
                                             AF.Identity, bias=tb)
                        nc.scalar.activation(Tflat[64:128, 1, sl], pst[64:128],
                                             AF.Identity, bias=tfbF[64:128])
                        nc.scalar.activation(Tflat[0:64, 1, sl], psf[0:64],
                                             AF.Identity, bias=fb)
                        nc.scalar.activation(Tflat[64:128, 0, sl], psf[64:128],
                                             AF.Identity, bias=tb2[64:128])

                    # row affinities: E[i, x] = sum_c f[c,y,i] t[c,y,x]
                    Wr4 = Wr.rearrange("i (h two) w -> i h two w", two=2)
                    Wc4 = Wc.rearrange("j (x two) y -> j x two y", two=2)
                    for y0 in range(0, H, 4):
                        psA = pe_ps.tile([96, 2, 96], F32, tag="peA")
                        psB = pe_ps.tile([96, 2, 96], F32, tag="peB")
                        for h in range(2):
                            nc.tensor.matmul(psA[:, h, :], F[:, y0 + 2 * h, :],
                                             T[:, y0 + 2 * h, :], start=True, stop=True)
                            nc.tensor.matmul(psB[:, h, :], F2[:, y0 + 2 * h + 1, :],
                                             T2[:, y0 + 2 * h + 1, :],
                                             start=True, stop=True)
                        h0 = y0 // 2
                        nc.scalar.activation(Wr4[:, h0:h0 + 2, 0, :], psA, AF.Exp)
                        nc.scalar.activation(Wr4[:, h0:h0 + 2, 1, :], psB, AF.Exp)
                    # col affinities: E[j, y] = sum_c f[c,j,x] t[c,y,x]; kill j==y
                    mb = bass.AP(tensor=mask.tensor, offset=mask.offset,
                                 ap=[mask.ap[0], [0, 4], mask.ap[1]])
                    mb2 = bass.AP(tensor=mask.tensor, offset=mask.offset,
                                  ap=[mask.ap[0], [0, 2], mask.ap[1]])
                    for x0 in range(0, W, 4):
                        psA = pe_ps.tile([96, 2, 96], F32, tag="peA")
                        psB = pe_ps.tile([96, 2, 96], F32, tag="peB")
                        for h in range(2):
                            nc.tensor.matmul(psA[:, h, :], F[:, :, x0 + 2 * h],
                                             T[:, :, x0 + 2 * h], start=True, stop=True)
                            nc.tensor.matmul(psB[:, h, :], F2[:, :, x0 + 2 * h + 1],
                                             T2[:, :, x0 + 2 * h + 1],
                                             start=True, stop=True)
                        h0 = x0 // 2
                        wcsA = Wc4[:, h0:h0 + 2, 0, :]
                        wcsB = Wc4[:, h0:h0 + 2, 1, :]
                        nc.scalar.activation(wcsA, psA, AF.Exp)
                        nc.scalar.activation(wcsB, psB, AF.Exp)
                        nc.gpsimd.tensor_mul(wcsA, wcsA, mb2)
                        nc.gpsimd.tensor_mul(wcsB, wcsB, mb2)

                WrT = Wr.rearrange("i h w -> i w h")
                WcT = Wc.rearrange("j x y -> j y x")
                Wrflat = Wr.rearrange("p h w -> p (h w)")

                # ---- phase 3a: col pass (first writer of U) ----
                U = p0.tile([128, KC, H, W], BF16, tag="big", name="U")
                with tc.tile_pool(name="pu1", bufs=4) as pu1:
                  with tc.tile_pool(name="pg_ps1", bufs=3, space="PSUM") as pg_ps1, \
                     tc.tile_pool(name="pd_ps1", bufs=2, space="PSUM") as pd_ps1, \
                     tc.tile_pool(name="pu_ps1", bufs=3, space="PSUM") as pu_ps1:
                    for x0 in range(0, W, 4):
                        xs = slice(x0, x0 + 4)
                        psd = pd_ps1.tile([128, 4, 96], F32, tag="pd")
                        nc.tensor.matmul(psd, ones96b, Wc[:, xs, :],
                                         start=True, stop=False)
                        nc.tensor.matmul(psd, ones96b, WrT[:, xs, :],
                                         start=False, stop=True)
                        rr = pu1.tile([128, 4, 96], BF16, tag="rr", bufs=3)
                        nc.scalar.activation(psd, psd, AF.Ln)
                        nc.scalar.activation(rr, psd, AF.Exp, scale=-1.0)
                        rrT = rr.rearrange("p x y -> p y x")
                        gts = []
                        for r in range(4):
                            psg = pg_ps1.tile([96, C_OUT], F32, tag="pg")
                            for k in range(KC):
                                nc.tensor.matmul(psg, Xbf[:, k, :, x0 + r],
                                                 gw[:, k, :],
                                                 start=(k == 0), stop=(k == KC - 1))
                            gt = pu1.tile([96, C_OUT], BF16, tag="gt", bufs=6)
                            nc.scalar.activation(gt, psg, AF.Copy)
                            gts.append(gt)
                        for cc in range(4):
                            psu = pu_ps1.tile([128, 4, 96], F32, tag="pu")
                            for r in range(4):
                                nc.tensor.matmul(
                                    psu[:, r, :],
                                    gts[r][:, cc * 128:(cc + 1) * 128],
                                    Wc[:, x0 + r, :], start=True, stop=True)
                            nc.vector.tensor_mul(
                                U[:, cc, :, x0:x0 + 4],
                                psu.rearrange("p x y -> p y x"), rrT)
                  # row-major denominators + normalize Wr
                  with tc.tile_pool(name="pdr_ps", bufs=6, space="PSUM") as pdr_ps:
                    for y0 in range(0, H, 4):
                        ys = slice(y0, y0 + 4)
                        sl4 = slice(y0 * 96, (y0 + 4) * 96)
                        psd2 = pdr_ps.tile([128, 4, 96], F32, tag="pd")
                        nc.tensor.matmul(psd2, ones96b, Wr[:, ys, :],
                                         start=True, stop=False)
                        nc.tensor.matmul(psd2, ones96b, WcT[:, ys, :],
                                         start=False, stop=True)
                        rw = pu1.tile([128, 4, 96], BF16, tag="rr", bufs=3)
                        nc.scalar.activation(psd2, psd2, AF.Ln)
                        nc.scalar.activation(rw, psd2, AF.Exp, scale=-1.0)
                        nc.gpsimd.tensor_mul(
                            Wrflat[:, sl4], Wrflat[:, sl4],
                            rw.rearrange("p a b -> p (a b)")[0:96, :])

            # ---- phase 3b/4: row pass + inc conv (overlapped) ----
            Uflat = U.rearrange("p a h w -> p a (h w)")
            with tc.tile_pool(name="pu2", bufs=4) as pu2, \
                 tc.tile_pool(name="pg_ps2", bufs=2, space="PSUM") as pg_ps2, \
                 tc.tile_pool(name="pu_ps2", bufs=2, space="PSUM") as pu_ps2, \
                 tc.tile_pool(name="pi", bufs=1) as pi, \
                 tc.tile_pool(name="pix", bufs=4) as pix, \
                 tc.tile_pool(name="po_ps", bufs=4, space="PSUM") as po_ps:
                incw = pi.tile([128, KC, C_IN], BF16)
                combb = pi.tile([128, KC], F32)
                nc.sync.dma_start(out=incw, in_=incw_d[:])
                nc.sync.dma_start(out=combb, in_=combb_d[:])
                def emit_inc_block(b):
                    sl = slice(b * 512, (b + 1) * 512)
                    for c2 in range(KC):
                        ps = po_ps.tile([128, 512], F32, tag="po")
                        for k in range(KC):
                            nc.tensor.matmul(ps, incw[:, k, c2 * 128:(c2 + 1) * 128],
                                             Uflat[:, k, sl],
                                             start=(k == 0), stop=(k == KC - 1))
                        xr = pix.tile([128, 512], F32, tag="xr")
                        nc.sync.dma_start(out=xr, in_=xf_d[c2][:, sl])
                        ot = pix.tile([128, 512], F32, tag="ot")
                        nc.vector.scalar_tensor_tensor(
                            ot, ps, combb[:, c2:c2 + 1], xr,
                            mybir.AluOpType.add, mybir.AluOpType.add)
                        nc.sync.dma_start(out=out_d[c2][:, sl], in_=ot)

                next_b = 0
                for y0 in range(0, H, 4):
                    gts = []
                    for r in range(4):
                        psg = pg_ps2.tile([96, C_OUT], F32, tag="pg")
                        for k in range(KC):
                            nc.tensor.matmul(psg, Xbf[:, k, y0 + r, :], gw[:, k, :],
                                             start=(k == 0), stop=(k == KC - 1))
                        gt = pu2.tile([96, C_OUT], BF16, tag="gt")
                        nc.scalar.activation(gt, psg, AF.Copy)
                        gts.append(gt)
                    for cc in range(4):
                        psu = pu_ps2.tile([128, 4 * 96], F32, tag="pu")
                        for r in range(4):
                            nc.tensor.matmul(
                                psu[:, r * 96:(r + 1) * 96],
                                gts[r][:, cc * 128:(cc + 1) * 128],
                                Wr[:, y0 + r, :], start=True, stop=True)
                        uv = U[:, cc, y0:y0 + 4, :]
                        nc.vector.tensor_add(
                            uv, uv, psu.rearrange("p (a b) -> p a b", a=4))
                    # emit inc blocks whose rows are already aggregated
                    while (next_b + 1) * 512 <= y0 * 96:
                        emit_inc_block(next_b)
                        next_b += 1
                while next_b < HW // 512:
                    emit_inc_block(next_b)
                    next_b += 1

    nc.finalize()
    return nc


def _prep_shared(t_w, t_b, f_w, f_b, g_w, g_b, inc_w, inc_b):
    bf = ml_dtypes.bfloat16
    tf_wT = np.concatenate([t_w.T, f_w.T], axis=1)  # (512, 128)
    d = {
        "tf_wT": np.ascontiguousarray(
            tf_wT.reshape(KC, 128, 128).transpose(1, 0, 2)).astype(bf),
        "tf_wT2": np.ascontiguousarray(
            np.concatenate([f_w.T, t_w.T], axis=1)
            .reshape(KC, 128, 128).transpose(1, 0, 2)).astype(bf),
        "g_wT": np.ascontiguousarray(
            g_w.T.reshape(KC, 128, C_OUT).transpose(1, 0, 2)).astype(bf),
        "inc_wT": np.ascontiguousarray(
            inc_w.T.reshape(KC, 128, C_IN).transpose(1, 0, 2)).astype(bf),
        "tf_b": np.concatenate([t_b, f_b]).reshape(128, 1).astype(np.float32),
        "tf_b2": np.concatenate([f_b, t_b]).reshape(128, 1).astype(np.float32),
        "comb_b": np.ascontiguousarray(
            (inc_b + inc_w @ g_b).reshape(KC, 128).T).astype(np.float32),
        "mask": (1.0 - np.eye(96)).astype(bf),
        "ones96b": np.ones((96, 128), bf),
    }
    return d


def _make_in_maps(inputs):
    x = np.asarray(inputs["x"], dtype=np.float32)
    shared = _prep_shared(*[
        np.asarray(inputs[k], np.float32)
        for k in ("t_w", "t_b", "f_w", "f_b", "g_w", "g_b", "inc_w", "inc_b")])
    bf = ml_dtypes.bfloat16
    in_maps = []
    for n in range(N):
        xi = x[n].reshape(KC, 128, HW)  # (4, 128, 9216)
        m = dict(shared)
        m["x_f32"] = np.ascontiguousarray(xi)
        m["x_bf"] = np.ascontiguousarray(xi.transpose(1, 0, 2)).astype(bf)
        in_maps.append(m)
    return in_maps


def kernel(x, t_w, t_b, f_w, f_b, g_w, g_b, inc_w, inc_b):
    in_maps = _make_in_maps(dict(
        x=x, t_w=t_w, t_b=t_b, f_w=f_w, f_b=f_b, g_w=g_w, g_b=g_b,
        inc_w=inc_w, inc_b=inc_b))

    if "nc" not in _cache:
        _cache["nc"] = build_program()
    res = run_bass_kernel_spmd(_cache["nc"], in_maps, core_ids=list(range(N)))
    out = np.stack([r["out"].reshape(C_IN, H, W) for r in res.results])
    return out.astype(np.float32)


if __name__ == "__main__":
    rng = np.random.default_rng(0)
    ins = {
        "x": rng.standard_normal((N, C_IN, H, W), dtype=np.float32),
        "t_w": rng.standard_normal((C_INNER, C_IN), dtype=np.float32) * 0.02,
        "t_b": np.zeros(C_INNER, np.float32),
        "f_w": rng.standard_normal((C_INNER, C_IN), dtype=np.float32) * 0.02,
        "f_b": np.zeros(C_INNER, np.float32),
        "g_w": rng.standard_normal((C_OUT, C_IN), dtype=np.float32) * 0.02,
        "g_b": np.zeros(C_OUT, np.float32),
        "inc_w": rng.standard_normal((C_IN, C_OUT), dtype=np.float32) * 0.02,
        "inc_b": np.zeros(C_IN, np.float32),
    }
    y = kernel(**ins)
    print(y.shape, y.dtype)



# revision 4
# speedup vs baseline: 1.3763x; 1.3763x over previous
"""Criss-cross attention (CCNet) kernel for 8 TRN2 NeuronCores.

Data-parallel over batch N=8: one image per core. Per image (512ch, 96x96):
  t/f = 1x1 conv to 64ch -> row/col affinities -> exp -> denominators ->
  reciprocal folded into Wr/Wc -> aggregate g' = (inc_w@g_w)@x along rows
  and cols -> two spatial-major outputs, recombined with the residual on
  host.

Key folds vs the reference:
  - inc_w @ g_w is precomputed on host (M), so the g conv and inc conv
    become ONE conv of the aggregated input (softmax weights sum to 1, so
    g_b folds into a host-side bias).
  - softmax normalization (1/denominator) is multiplied into the attention
    weights before aggregation, so aggregation outputs are final.
  - residual + biases are added on host; outputs ship as bf16.
"""

import sys

sys.path.insert(0, "/opt/trn_rl_repo")

from contextlib import ExitStack

import numpy as np
import ml_dtypes

import concourse.bass as bass
import concourse.bacc as bacc
import concourse.tile as tile
from concourse import mybir
from concourse.bass_utils import run_bass_kernel_spmd

BF16 = mybir.dt.bfloat16
F32 = mybir.dt.float32
AF = mybir.ActivationFunctionType

N, C_IN, C_INNER, C_OUT, H, W = 8, 512, 64, 512, 96, 96
HW = H * W  # 9216
KC = C_IN // 128  # 4 contraction chunks

_cache = {}


def build_program():
    nc = bacc.Bacc()

    # ---- DRAM I/O ----
    xbf_d = nc.dram_tensor("x_bf", (128, KC, HW), BF16, kind="ExternalInput")
    tfw_d = nc.dram_tensor("tf_wT", (128, KC, 128), BF16, kind="ExternalInput")
    tfw2_d = nc.dram_tensor("tf_wT2", (128, KC, 128), BF16, kind="ExternalInput")
    mw_d = nc.dram_tensor("m_wT", (128, KC, C_OUT), BF16, kind="ExternalInput")
    tfb_d = nc.dram_tensor("tf_b", (128, 1), F32, kind="ExternalInput")
    tfb2_d = nc.dram_tensor("tf_b2", (128, 1), F32, kind="ExternalInput")
    mask_d = nc.dram_tensor("mask", (96, 96), BF16, kind="ExternalInput")
    ones96b_d = nc.dram_tensor("ones96b", (96, 128), BF16, kind="ExternalInput")
    ocol_d = nc.dram_tensor("out_col", (KC, 128, HW), BF16, kind="ExternalOutput")
    orow_d = nc.dram_tensor("out_row", (KC, 128, HW), BF16, kind="ExternalOutput")

    with ExitStack() as ctx:
        tc = ctx.enter_context(tile.TileContext(nc))
        p0 = ctx.enter_context(tc.tile_pool(name="p0", bufs=1))

        # ---- persistent tiles ----
        Xbf = p0.tile([128, KC, H, W], BF16)  # channel-major image, bf16
        ones96b = p0.tile([96, 128], BF16)
        mask = p0.tile([96, 96], BF16)
        mw = p0.tile([128, KC, C_OUT], BF16)
        TFb = p0.tile([128, 2, H, W], BF16)  # t/f activations (doubled)
        Wr = p0.tile([96, H, W], BF16)       # exp row affin: Wr[i, y, x]
        Wc = p0.tile([96, W, H], BF16)       # exp col affin: Wc[j, x, y]
        rr = p0.tile([128, W, H], BF16)      # 1/denominator, [*, x, y] bcast

        nc.sync.dma_start(out=ones96b, in_=ones96b_d[:])
        nc.sync.dma_start(out=mask, in_=mask_d[:])
        nc.sync.dma_start(out=mw, in_=mw_d[:])
        xv = xbf_d[:].rearrange("p a (h w) -> p a h w", h=H)

        T = TFb[0:64, 0]
        F = TFb[0:64, 1]
        T2 = TFb[64:128, 0]
        F2 = TFb[64:128, 1]

        # ---- phase 1: t/f conv ----
        with tc.tile_pool(name="pe", bufs=1) as pe, \
             tc.tile_pool(name="ptf_ps", bufs=2, space="PSUM") as ptf_ps:
            tfw = pe.tile([128, KC, 128], BF16)
            tfw2 = pe.tile([128, KC, 128], BF16)
            tb = pe.tile([64, 1], F32)
            fb = pe.tile([64, 1], F32)
            tfbF = pe.tile([128, 1], F32)
            tb2 = pe.tile([128, 1], F32)
            nc.sync.dma_start(out=tfw, in_=tfw_d[:])
            nc.sync.dma_start(out=tfw2, in_=tfw2_d[:])
            nc.sync.dma_start(out=tb, in_=tfb_d[0:64])
            nc.sync.dma_start(out=fb, in_=tfb_d[64:128])
            nc.sync.dma_start(out=tfbF, in_=tfb_d[:])
            nc.sync.dma_start(out=tb2, in_=tfb2_d[:])
            for q in range(8):
                for k in range(KC):
                    nc.sync.dma_start(
                        out=Xbf[:, k, q * 12:(q + 1) * 12, :],
                        in_=xv[:, k, q * 12:(q + 1) * 12, :])

            Xflat = Xbf.rearrange("p a h w -> p a (h w)")
            Tflat = TFb.rearrange("p c h w -> p c (h w)")
            for b in range(HW // 512):
                sl = slice(b * 512, (b + 1) * 512)
                pst = ptf_ps.tile([128, 512], F32, tag="pt")
                psf = ptf_ps.tile([128, 512], F32, tag="pf")
                for k in range(KC):
                    nc.tensor.matmul(
                        pst, tfw[:, k, :], Xflat[:, k, sl],
                        start=(k == 0), stop=(k == KC - 1))
                for k in range(KC):
                    nc.tensor.matmul(
                        psf, tfw2[:, k, :], Xflat[:, k, sl],
                        start=(k == 0), stop=(k == KC - 1))
                # pst = [t; f] stacked, psf = [f; t] stacked
                nc.scalar.activation(Tflat[0:64, 0, sl], pst[0:64],
                                     AF.Identity, bias=tb)
                nc.vector.tensor_scalar_add(Tflat[64:128, 1, sl], pst[64:128],
                                            tfbF[64:128])
                nc.scalar.activation(Tflat[0:64, 1, sl], psf[0:64],
                                     AF.Identity, bias=fb)
                nc.vector.tensor_scalar_add(Tflat[64:128, 0, sl], psf[64:128],
                                            tb2[64:128])

        # ---- phase 2: affinities + exp (+ mask on cols) ----
        # Wr[i, y, x] = exp(sum_c f[c,y,i] t[c,y,x]); 8-row blocks.
        Wr8 = Wr.rearrange("i (b four two) w -> i b four two w", four=4, two=2)
        Wc8 = Wc.rearrange("j (b four two) y -> j b four two y", four=4, two=2)
        with tc.tile_pool(name="pe_ps", bufs=2, space="PSUM") as pe_ps:
            for y0 in range(0, H, 8):
                psA = pe_ps.tile([96, 4, 96], F32, tag="peA")
                psB = pe_ps.tile([96, 4, 96], F32, tag="peB")
                for h in range(4):
                    nc.tensor.matmul(psA[:, h, :], F[:, y0 + 2 * h, :],
                                     T[:, y0 + 2 * h, :], start=True, stop=True)
                    nc.tensor.matmul(psB[:, h, :], F2[:, y0 + 2 * h + 1, :],
                                     T2[:, y0 + 2 * h + 1, :],
                                     start=True, stop=True)
                b = y0 // 8
                nc.scalar.activation(Wr8[:, b, :, 0, :], psA, AF.Exp)
                nc.scalar.activation(Wr8[:, b, :, 1, :], psB, AF.Exp)
            # col affinities: Wc[j, x, y] = exp(sum_c f[c,j,x] t[c,y,x]), j!=y
            mb8 = bass.AP(tensor=mask.tensor, offset=mask.offset,
                          ap=[mask.ap[0], [0, 8], mask.ap[1]])
            for x0 in range(0, W, 8):
                psA = pe_ps.tile([96, 4, 96], F32, tag="peA")
                psB = pe_ps.tile([96, 4, 96], F32, tag="peB")
                for h in range(4):
                    nc.tensor.matmul(psA[:, h, :], F[:, :, x0 + 2 * h],
                                     T[:, :, x0 + 2 * h], start=True, stop=True)
                    nc.tensor.matmul(psB[:, h, :], F2[:, :, x0 + 2 * h + 1],
                                     T2[:, :, x0 + 2 * h + 1],
                                     start=True, stop=True)
                b = x0 // 8
                nc.scalar.activation(Wc8[:, b, :, 0, :], psA, AF.Exp)
                nc.scalar.activation(Wc8[:, b, :, 1, :], psB, AF.Exp)
                wcs = Wc[:, x0:x0 + 8, :]
                nc.gpsimd.tensor_mul(wcs, wcs, mb8)

        # ---- phase 3: denominators ([x, y] layout) -> 1/D -> fold into W ----
        WrT = Wr.rearrange("i h w -> i w h")
        with tc.tile_pool(name="pd_ps", bufs=2, space="PSUM") as pd_ps:
            for x0 in range(0, W, 4):
                xs = slice(x0, x0 + 4)
                psd = pd_ps.tile([128, 4, 96], F32, tag="pd")
                nc.tensor.matmul(psd, ones96b, Wc[:, xs, :],
                                 start=True, stop=False)
                nc.tensor.matmul(psd, ones96b, WrT[:, xs, :],
                                 start=False, stop=True)
                with nc.allow_low_precision("softmax denom scale, bf16 ok"):
                    nc.vector.reciprocal(rr[:, xs, :], psd)
            # normalize: Wc[j, x, y] *= rr[x, y]; Wr[i, y, x] *= rr[x, y]
            rrv = rr[0:96]
            rrT = rrv.rearrange("p x y -> p y x")
            for x0 in range(0, W, 8):
                xs = slice(x0, x0 + 8)
                nc.gpsimd.tensor_mul(Wc[:, xs, :], Wc[:, xs, :], rrv[:, xs, :])
            for y0 in range(0, H, 8):
                ys = slice(y0, y0 + 8)
                nc.gpsimd.tensor_mul(Wr[:, ys, :], Wr[:, ys, :], rrT[:, ys, :])

        # ---- phases 4/5: conv g'=(inc_w@g_w)@x + aggregate, col then row ----
        ocol_v = ocol_d[:].rearrange("a p hw -> p a hw")
        orow_v = orow_d[:].rearrange("a p hw -> p a hw")
        with tc.tile_pool(name="pg", bufs=8) as pg, \
             tc.tile_pool(name="pu", bufs=3) as pu, \
             tc.tile_pool(name="pg_ps", bufs=3, space="PSUM") as pg_ps, \
             tc.tile_pool(name="pu_ps", bufs=3, space="PSUM") as pu_ps:
            for mode in ("col", "row"):
                Wa = Wc if mode == "col" else Wr
                out_v = ocol_v if mode == "col" else orow_v
                for s0 in range(0, 96, 4):
                    # g' conv for 4 lines (cols x=s0.. or rows y=s0..)
                    gts = []
                    for r in range(4):
                        psg = pg_ps.tile([96, C_OUT], F32, tag="pg")
                        for k in range(KC):
                            lhs = (Xbf[:, k, :, s0 + r] if mode == "col"
                                   else Xbf[:, k, s0 + r, :])
                            nc.tensor.matmul(psg, lhs, mw[:, k, :],
                                             start=(k == 0), stop=(k == KC - 1))
                        gt = pg.tile([96, C_OUT], BF16, tag="gt")
                        if r % 2 == 0:
                            nc.scalar.copy(gt, psg)
                        else:
                            nc.vector.tensor_copy(gt, psg)
                        gts.append(gt)
                    # aggregation: uc[o, cc, r, s] final (weights pre-normalized)
                    uc = pu.tile([128, KC, 4, 96], BF16, tag="uc")
                    for cc in range(KC):
                        psu = pu_ps.tile([128, 4, 96], F32, tag="pu")
                        for r in range(4):
                            nc.tensor.matmul(
                                psu[:, r, :],
                                gts[r][:, cc * 128:(cc + 1) * 128],
                                Wa[:, s0 + r, :], start=True, stop=True)
                        if cc % 2 == 0:
                            nc.vector.tensor_copy(uc[:, cc], psu)
                        else:
                            nc.scalar.copy(uc[:, cc], psu)
                    nc.sync.dma_start(
                        out=out_v[:, :, s0 * 96:(s0 + 4) * 96],
                        in_=uc.rearrange("p a r s -> p a (r s)"))

    nc.finalize()
    return nc


def _prep_shared(t_w, t_b, f_w, f_b, g_w, g_b, inc_w, inc_b):
    bf = ml_dtypes.bfloat16
    m_w = inc_w @ g_w  # (C_IN, C_IN) fold: inc(g(.)) == M @ .
    d = {
        "tf_wT": np.ascontiguousarray(
            np.concatenate([t_w.T, f_w.T], axis=1)
            .reshape(KC, 128, 128).transpose(1, 0, 2)).astype(bf),
        "tf_wT2": np.ascontiguousarray(
            np.concatenate([f_w.T, t_w.T], axis=1)
            .reshape(KC, 128, 128).transpose(1, 0, 2)).astype(bf),
        "m_wT": np.ascontiguousarray(
            m_w.T.reshape(KC, 128, C_OUT).transpose(1, 0, 2)).astype(bf),
        "tf_b": np.concatenate([t_b, f_b]).reshape(128, 1).astype(np.float32),
        "tf_b2": np.concatenate([f_b, t_b]).reshape(128, 1).astype(np.float32),
        "mask": (1.0 - np.eye(96)).astype(bf),
        "ones96b": np.ones((96, 128), bf),
    }
    comb_b = (inc_b + inc_w @ g_b).astype(np.float32)  # host-side bias
    return d, comb_b


def _make_in_maps(inputs):
    x = np.asarray(inputs["x"], dtype=np.float32)
    shared, comb_b = _prep_shared(*[
        np.asarray(inputs[k], np.float32)
        for k in ("t_w", "t_b", "f_w", "f_b", "g_w", "g_b", "inc_w", "inc_b")])
    _cache["comb_b"] = comb_b
    _cache["x"] = x
    bf = ml_dtypes.bfloat16
    in_maps = []
    for n in range(N):
        xi = x[n].reshape(KC, 128, HW)  # (4, 128, 9216)
        m = dict(shared)
        m["x_bf"] = np.ascontiguousarray(xi.transpose(1, 0, 2)).astype(bf)
        in_maps.append(m)
    return in_maps


def _post(results):
    x = _cache["x"]
    comb_b = _cache["comb_b"]
    out = np.empty((N, C_IN, H, W), np.float32)
    for n in range(N):
        row = results[n]["out_row"].astype(np.float32).reshape(C_IN, H, W)
        col = results[n]["out_col"].astype(np.float32).reshape(C_IN, W, H)
        out[n] = x[n] + comb_b[:, None, None] + row + col.transpose(0, 2, 1)
    return out


def kernel(x, t_w, t_b, f_w, f_b, g_w, g_b, inc_w, inc_b):
    in_maps = _make_in_maps(dict(
        x=x, t_w=t_w, t_b=t_b, f_w=f_w, f_b=f_b, g_w=g_w, g_b=g_b,
        inc_w=inc_w, inc_b=inc_b))

    if "nc" not in _cache:
        _cache["nc"] = build_program()
    res = run_bass_kernel_spmd(_cache["nc"], in_maps, core_ids=list(range(N)))
    return _post(res.results)


if __name__ == "__main__":
    rng = np.random.default_rng(0)
    ins = {
        "x": rng.standard_normal((N, C_IN, H, W), dtype=np.float32),
        "t_w": rng.standard_normal((C_INNER, C_IN), dtype=np.float32) * 0.02,
        "t_b": np.zeros(C_INNER, np.float32),
        "f_w": rng.standard_normal((C_INNER, C_IN), dtype=np.float32) * 0.02,
        "f_b": np.zeros(C_INNER, np.float32),
        "g_w": rng.standard_normal((C_OUT, C_IN), dtype=np.float32) * 0.02,
        "g_b": np.zeros(C_OUT, np.float32),
        "inc_w": rng.standard_normal((C_IN, C_OUT), dtype=np.float32) * 0.02,
        "inc_b": np.zeros(C_IN, np.float32),
    }
    y = kernel(**ins)
    print(y.shape, y.dtype)


# revision 6
# speedup vs baseline: 1.6495x; 1.1985x over previous
"""Criss-cross attention (CCNet) kernel for 8 TRN2 NeuronCores.

Data-parallel over batch N=8: one image per core. Per image (512ch, 96x96):
  t/f = 1x1 conv to 64ch -> row/col affinities -> exp -> denominators ->
  reciprocal folded into Wr/Wc -> aggregate g' = (inc_w@g_w)@x along rows
  and cols -> two spatial-major outputs, recombined with the residual on
  host.

Key folds vs the reference:
  - inc_w @ g_w is precomputed on host (M), so the g conv and inc conv
    become ONE conv of the aggregated input (softmax weights sum to 1, so
    g_b folds into a host-side bias).
  - The M conv runs in fp8e4 DoubleRow (2 contraction chunks per matmul);
    M is prescaled by 512 on host, outputs divided by 512 on host.
  - softmax normalization (1/denominator) is multiplied into the attention
    weights before aggregation, so aggregation outputs are final.
  - Aggregation keeps the attention line stationary and streams g' (one
    N=512 matmul per line), producing spatial-major outputs.
  - residual + biases are added on host; outputs ship as bf16.
"""

import sys

sys.path.insert(0, "/opt/trn_rl_repo")

from contextlib import ExitStack

import numpy as np
import ml_dtypes

import concourse.bass as bass
import concourse.bacc as bacc
import concourse.tile as tile
from concourse import mybir
from concourse.bass_utils import run_bass_kernel_spmd

BF16 = mybir.dt.bfloat16
F32 = mybir.dt.float32
FP8 = mybir.dt.float8e4
AF = mybir.ActivationFunctionType
DR = mybir.MatmulPerfMode.DoubleRow

N, C_IN, C_INNER, C_OUT, H, W = 8, 512, 64, 512, 96, 96
HW = H * W  # 9216
KC = C_IN // 128  # 4 contraction chunks
MSCALE = 512.0  # fp8 prescale of M = inc_w @ g_w

_cache = {}


def build_program():
    nc = bacc.Bacc()

    # ---- DRAM I/O ----
    xbf_d = nc.dram_tensor("x_bf", (128, KC, HW), BF16, kind="ExternalInput")
    xf8_d = nc.dram_tensor("x_f8", (128, KC, HW), FP8, kind="ExternalInput")
    tfw_d = nc.dram_tensor("tf_wT", (128, KC, 128), BF16, kind="ExternalInput")
    tfw2_d = nc.dram_tensor("tf_wT2", (128, KC, 128), BF16, kind="ExternalInput")
    mw_d = nc.dram_tensor("m_wT", (128, KC, C_OUT), FP8, kind="ExternalInput")
    tfb_d = nc.dram_tensor("tf_b", (128, 1), F32, kind="ExternalInput")
    tfb2_d = nc.dram_tensor("tf_b2", (128, 1), F32, kind="ExternalInput")
    mask_d = nc.dram_tensor("mask", (96, 96), BF16, kind="ExternalInput")
    ones96b_d = nc.dram_tensor("ones96b", (96, 128), BF16, kind="ExternalInput")
    # spatial-major outputs: [line, pos-in-line, channel]
    ocol_d = nc.dram_tensor("out_col", (W, H, C_OUT), BF16, kind="ExternalOutput")
    orow_d = nc.dram_tensor("out_row", (H, W, C_OUT), BF16, kind="ExternalOutput")

    with ExitStack() as ctx:
        tc = ctx.enter_context(tile.TileContext(nc))
        p0 = ctx.enter_context(tc.tile_pool(name="p0", bufs=1))

        # ---- persistent tiles ----
        Xbf = p0.tile([128, KC, H, W], BF16)  # channel-major image, bf16
        ones96b = p0.tile([96, 128], BF16)
        mask = p0.tile([96, 96], BF16)
        mw = p0.tile([128, KC, C_OUT], FP8)
        TFb = p0.tile([128, 2, H, W], BF16, tag="big", name="TFb")
        Wr = p0.tile([96, H, W], BF16)       # exp row affin: Wr[i, y, x]
        Wc = p0.tile([96, W, H], BF16)       # exp col affin: Wc[j, x, y]
        rr = p0.tile([128, W, H], BF16)      # 1/denominator, [*, x, y] bcast

        nc.sync.dma_start(out=ones96b, in_=ones96b_d[:])
        nc.sync.dma_start(out=mask, in_=mask_d[:])
        nc.sync.dma_start(out=mw, in_=mw_d[:])
        xv = xbf_d[:].rearrange("p a (h w) -> p a h w", h=H)
        xv8 = xf8_d[:].rearrange("p a (h w) -> p a h w", h=H)

        T = TFb[0:64, 0]
        F = TFb[0:64, 1]
        T2 = TFb[64:128, 0]
        F2 = TFb[64:128, 1]

        # ---- phase 1: t/f conv ----
        with tc.tile_pool(name="pe", bufs=1) as pe, \
             tc.tile_pool(name="ptf_ps", bufs=2, space="PSUM") as ptf_ps:
            tfw = pe.tile([128, KC, 128], BF16)
            tfw2 = pe.tile([128, KC, 128], BF16)
            tb = pe.tile([64, 1], F32)
            fb = pe.tile([64, 1], F32)
            tfbF = pe.tile([128, 1], F32)
            tb2 = pe.tile([128, 1], F32)
            nc.sync.dma_start(out=tfw, in_=tfw_d[:])
            nc.sync.dma_start(out=tfw2, in_=tfw2_d[:])
            nc.sync.dma_start(out=tb, in_=tfb_d[0:64])
            nc.sync.dma_start(out=fb, in_=tfb_d[64:128])
            nc.sync.dma_start(out=tfbF, in_=tfb_d[:])
            nc.sync.dma_start(out=tb2, in_=tfb2_d[:])
            for q in range(8):
                for k in range(KC):
                    nc.sync.dma_start(
                        out=Xbf[:, k, q * 12:(q + 1) * 12, :],
                        in_=xv[:, k, q * 12:(q + 1) * 12, :])

            Xflat = Xbf.rearrange("p a h w -> p a (h w)")
            Tflat = TFb.rearrange("p c h w -> p c (h w)")
            for b in range(HW // 512):
                sl = slice(b * 512, (b + 1) * 512)
                pst = ptf_ps.tile([128, 512], F32, tag="pt")
                psf = ptf_ps.tile([128, 512], F32, tag="pf")
                for k in range(KC):
                    nc.tensor.matmul(
                        pst, tfw[:, k, :], Xflat[:, k, sl],
                        start=(k == 0), stop=(k == KC - 1))
                for k in range(KC):
                    nc.tensor.matmul(
                        psf, tfw2[:, k, :], Xflat[:, k, sl],
                        start=(k == 0), stop=(k == KC - 1))
                # pst = [t; f] stacked, psf = [f; t] stacked
                nc.scalar.activation(Tflat[0:64, 0, sl], pst[0:64],
                                     AF.Identity, bias=tb)
                nc.vector.tensor_scalar_add(Tflat[64:128, 1, sl], pst[64:128],
                                            tfbF[64:128])
                nc.scalar.activation(Tflat[0:64, 1, sl], psf[0:64],
                                     AF.Identity, bias=fb)
                nc.vector.tensor_scalar_add(Tflat[64:128, 0, sl], psf[64:128],
                                            tb2[64:128])

        # ---- phase 2: affinities + exp (+ mask on cols) ----
        # Wr[i, y, x] = exp(sum_c f[c,y,i] t[c,y,x]); 8-row blocks.
        Wr8 = Wr.rearrange("i (b four two) w -> i b four two w", four=4, two=2)
        Wc8 = Wc.rearrange("j (b four two) y -> j b four two y", four=4, two=2)
        with tc.tile_pool(name="pe_ps", bufs=2, space="PSUM") as pe_ps:
            for y0 in range(0, H, 8):
                psA = pe_ps.tile([96, 4, 96], F32, tag="peA")
                psB = pe_ps.tile([96, 4, 96], F32, tag="peB")
                for h in range(4):
                    nc.tensor.matmul(psA[:, h, :], F[:, y0 + 2 * h, :],
                                     T[:, y0 + 2 * h, :], start=True, stop=True)
                    nc.tensor.matmul(psB[:, h, :], F2[:, y0 + 2 * h + 1, :],
                                     T2[:, y0 + 2 * h + 1, :],
                                     start=True, stop=True)
                b = y0 // 8
                nc.scalar.activation(Wr8[:, b, :, 0, :], psA, AF.Exp)
                nc.scalar.activation(Wr8[:, b, :, 1, :], psB, AF.Exp)
            # col affinities: Wc[j, x, y] = exp(sum_c f[c,j,x] t[c,y,x]), j!=y
            mb8 = bass.AP(tensor=mask.tensor, offset=mask.offset,
                          ap=[mask.ap[0], [0, 8], mask.ap[1]])
            for x0 in range(0, W, 8):
                psA = pe_ps.tile([96, 4, 96], F32, tag="peA")
                psB = pe_ps.tile([96, 4, 96], F32, tag="peB")
                for h in range(4):
                    nc.tensor.matmul(psA[:, h, :], F[:, :, x0 + 2 * h],
                                     T[:, :, x0 + 2 * h], start=True, stop=True)
                    nc.tensor.matmul(psB[:, h, :], F2[:, :, x0 + 2 * h + 1],
                                     T2[:, :, x0 + 2 * h + 1],
                                     start=True, stop=True)
                b = x0 // 8
                nc.scalar.activation(Wc8[:, b, :, 0, :], psA, AF.Exp)
                nc.scalar.activation(Wc8[:, b, :, 1, :], psB, AF.Exp)
                wcs = Wc[:, x0:x0 + 8, :]
                nc.gpsimd.tensor_mul(wcs, wcs, mb8)

        # fp8 image for the g' conv reuses the t/f slot (disjoint lifetime)
        Xf8 = p0.tile([128, KC, H, W], FP8, tag="big", name="Xf8")
        for q in range(8):
            for k in range(KC):
                nc.sync.dma_start(
                    out=Xf8[:, k, q * 12:(q + 1) * 12, :],
                    in_=xv8[:, k, q * 12:(q + 1) * 12, :])

        # ---- phase 3: denominators ([x, y] layout) -> 1/D -> fold into W ----
        WrT = Wr.rearrange("i h w -> i w h")
        with tc.tile_pool(name="pd_ps", bufs=2, space="PSUM") as pd_ps, \
             tc.tile_pool(name="prf", bufs=2) as prf:
            for x0 in range(0, W, 4):
                xs = slice(x0, x0 + 4)
                psd = pd_ps.tile([128, 4, 96], F32, tag="pd")
                nc.tensor.matmul(psd, ones96b, Wc[:, xs, :],
                                 start=True, stop=False)
                nc.tensor.matmul(psd, ones96b, WrT[:, xs, :],
                                 start=False, stop=True)
                rf = prf.tile([128, 4, 96], F32, tag="rf")
                nc.vector.reciprocal_approx_fast(out=rf, in_=psd)
                nc.vector.tensor_copy(rr[:, xs, :], rf)
            # normalize: Wc[j, x, y] *= rr[x, y]; Wr[i, y, x] *= rr[x, y]
            rrv = rr[0:96]
            rrT = rrv.rearrange("p x y -> p y x")
            for x0 in range(0, W, 8):
                xs = slice(x0, x0 + 8)
                nc.gpsimd.tensor_mul(Wc[:, xs, :], Wc[:, xs, :], rrv[:, xs, :])
            for y0 in range(0, H, 8):
                ys = slice(y0, y0 + 8)
                nc.gpsimd.tensor_mul(Wr[:, ys, :], Wr[:, ys, :], rrT[:, ys, :])

        # ---- phases 4/5: conv g'=(M@x) fp8-DoubleRow + aggregate ----
        with tc.tile_pool(name="pg", bufs=4) as pg, \
             tc.tile_pool(name="pu", bufs=3) as pu, \
             tc.tile_pool(name="pg_ps", bufs=2, space="PSUM") as pg_ps, \
             tc.tile_pool(name="pu_ps", bufs=2, space="PSUM") as pu_ps:
            for mode in ("col", "row"):
                Wa = Wc if mode == "col" else Wr
                out_d = ocol_d if mode == "col" else orow_d
                for s0 in range(0, 96, 2):
                    # g' conv for 2 lines -> psg2[:, i, :]
                    psg2 = pg_ps.tile([96, 2, C_OUT], F32, tag="pg")
                    for i in range(2):
                        for k in range(2):
                            lhs = (Xf8[:, 2 * k:2 * k + 2, :, s0 + i]
                                   if mode == "col"
                                   else Xf8[:, 2 * k:2 * k + 2, s0 + i, :])
                            nc.tensor.matmul(psg2[:, i, :], lhs,
                                             mw[:, 2 * k:2 * k + 2, :],
                                             start=(k == 0), stop=(k == 1),
                                             perf_mode=DR)
                    gt = pg.tile([96, 2, C_OUT], BF16, tag="gt")
                    if (s0 // 2) % 2 == 0:
                        nc.scalar.copy(gt, psg2)
                    else:
                        nc.vector.tensor_copy(gt, psg2)
                    # aggregation: psu2[:, i, :] = Wa[:, s0+i, :].T @ gt[:, i, :]
                    psu2 = pu_ps.tile([96, 2, C_OUT], F32, tag="pu")
                    for i in range(2):
                        nc.tensor.matmul(psu2[:, i, :], Wa[:, s0 + i, :],
                                         gt[:, i, :], start=True, stop=True)
                    uc = pu.tile([96, 2, C_OUT], BF16, tag="uc")
                    if (s0 // 2) % 2 == 1:
                        nc.scalar.copy(uc, psu2)
                    else:
                        nc.vector.tensor_copy(uc, psu2)
                    nc.sync.dma_start(
                        out=out_d[s0:s0 + 2].rearrange("l p c -> p l c"),
                        in_=uc)

    nc.finalize()
    return nc


def _prep_shared(t_w, t_b, f_w, f_b, g_w, g_b, inc_w, inc_b):
    bf = ml_dtypes.bfloat16
    f8 = ml_dtypes.float8_e4m3
    m_w = inc_w @ g_w  # (C_IN, C_IN) fold: inc(g(.)) == M @ .
    d = {
        "tf_wT": np.ascontiguousarray(
            np.concatenate([t_w.T, f_w.T], axis=1)
            .reshape(KC, 128, 128).transpose(1, 0, 2)).astype(bf),
        "tf_wT2": np.ascontiguousarray(
            np.concatenate([f_w.T, t_w.T], axis=1)
            .reshape(KC, 128, 128).transpose(1, 0, 2)).astype(bf),
        "m_wT": np.ascontiguousarray(
            (m_w.T * MSCALE).reshape(KC, 128, C_OUT)
            .transpose(1, 0, 2)).astype(f8),
        "tf_b": np.concatenate([t_b, f_b]).reshape(128, 1).astype(np.float32),
        "tf_b2": np.concatenate([f_b, t_b]).reshape(128, 1).astype(np.float32),
        "mask": (1.0 - np.eye(96)).astype(bf),
        "ones96b": np.ones((96, 128), bf),
    }
    comb_b = (inc_b + inc_w @ g_b).astype(np.float32)  # host-side bias
    return d, comb_b


def _make_in_maps(inputs):
    x = np.asarray(inputs["x"], dtype=np.float32)
    shared, comb_b = _prep_shared(*[
        np.asarray(inputs[k], np.float32)
        for k in ("t_w", "t_b", "f_w", "f_b", "g_w", "g_b", "inc_w", "inc_b")])
    _cache["comb_b"] = comb_b
    _cache["x"] = x
    bf = ml_dtypes.bfloat16
    f8 = ml_dtypes.float8_e4m3
    in_maps = []
    for n in range(N):
        xi = x[n].reshape(KC, 128, HW)  # (4, 128, 9216)
        xt = np.ascontiguousarray(xi.transpose(1, 0, 2))
        m = dict(shared)
        m["x_bf"] = xt.astype(bf)
        m["x_f8"] = xt.astype(f8)
        in_maps.append(m)
    return in_maps


def _post(results):
    x = _cache["x"]
    comb_b = _cache["comb_b"]
    inv = 1.0 / MSCALE
    out = np.empty((N, C_IN, H, W), np.float32)
    for n in range(N):
        row = results[n]["out_row"].astype(np.float32)  # (H, W, C)
        col = results[n]["out_col"].astype(np.float32)  # (W, H, C)
        agg = row.transpose(2, 0, 1) + col.transpose(2, 1, 0)
        out[n] = x[n] + comb_b[:, None, None] + agg * inv
    return out


def kernel(x, t_w, t_b, f_w, f_b, g_w, g_b, inc_w, inc_b):
    in_maps = _make_in_maps(dict(
        x=x, t_w=t_w, t_b=t_b, f_w=f_w, f_b=f_b, g_w=g_w, g_b=g_b,
        inc_w=inc_w, inc_b=inc_b))

    if "nc" not in _cache:
        _cache["nc"] = build_program()
    res = run_bass_kernel_spmd(_cache["nc"], in_maps, core_ids=list(range(N)))
    return _post(res.results)


if __name__ == "__main__":
    rng = np.random.default_rng(0)
    ins = {
        "x": rng.standard_normal((N, C_IN, H, W), dtype=np.float32),
        "t_w": rng.standard_normal((C_INNER, C_IN), dtype=np.float32) * 0.02,
        "t_b": np.zeros(C_INNER, np.float32),
        "f_w": rng.standard_normal((C_INNER, C_IN), dtype=np.float32) * 0.02,
        "f_b": np.zeros(C_INNER, np.float32),
        "g_w": rng.standard_normal((C_OUT, C_IN), dtype=np.float32) * 0.02,
        "g_b": np.zeros(C_OUT, np.float32),
        "inc_w": rng.standard_normal((C_IN, C_OUT), dtype=np.float32) * 0.02,
        "inc_b": np.zeros(C_IN, np.float32),
    }
    y = kernel(**ins)
    print(y.shape, y.dtype)
